# revision 13
# baseline (speedup 1.0000x reference)
"""Trainium2 Bass kernel for nn_ContrastLoss (8-core SPMD).

Math (reference):
    z1 = x1/||x1||, z2 = x2/||x2||                    [8192, 512]
    S = exp(z1 @ z2.T / tau)                          [8192, 8192]
    nts = exp(sum(pro*x1, -1) / tau)                  [8192]
    fenzi = nts + sum(S*gat, -1); fenmu = nts + sum(S, -1)
    loss = mean(-log(fenzi / (fenmu + 1e-8)))

Sharding: 4x2 grid over (rows of x1, rows of x2). Each core computes its
[2048, 4096] block of S on the fly (never materialized), producing partial
row sums. Host sums partials over the x2 axis and does the final tiny
[8192]-row combine in float32 (faithful to the reference's fp32 overflow:
exp(dot/tau) hits inf for ~500 rows, so the reference mean is NaN, which
this kernel reproduces exactly).

On-device per core:
  - x1/x2 row norms via fused DVE tensor_tensor_reduce (sumsq)
  - r = exp(-0.5*ln(sumsq)) on ACT (ln+exp share one table set)
  - normalization folded into PE transpose-with-diag(r): x1T is scaled by
    1/(tau*||x1_m||), z2T by 1/||x2_n||, so matmul output is directly
    z1.z2/tau
  - main loop: fp32 matmul (K=512 as 4 accumulating MMs) -> ACT exp with
    accum_out (plain row sum for free) -> DVE tensor_tensor_reduce with
    gat tile (weighted row sum, fused multiply+reduce)
"""

import math

import numpy as np

N = 8192
H = 512
TAU = 0.5
EPS_DEN = 1e-8
GM, GN = 4, 2          # core grid: GM row-shards x GN col-shards
NC = GM * GN
MS = N // GM           # rows of x1 per core (2048)
NS = N // GN           # rows of x2 per core (4096)
T1 = MS // 128         # x1 tiles per core (16)
T2 = NS // 128         # x2 tiles per core (32)
NJ = NS // 512         # 512-wide n-chunks per core (8)

_CACHE = {}


_SPLIT_CTR = [0]


def _split_multi_waits(nc):
    """This container's walrus rejects any instruction carrying more than one
    semaphore wait ("Too many sync wait commands"). Tile freely attaches
    several. Move extra waits onto NoOp instructions inserted just before the
    carrier (same engine) — equivalent, since all waits are monotonic sem-ge
    conditions checked by a serial engine."""
    import bass_rust
    from concourse import mybir

    for f in nc.m.functions:
        for bb in f.blocks:
            out = []
            changed = False
            for inst in bb.instructions:
                si = getattr(inst, "sync_info", None)
                waits = list(si.on_wait) if si is not None else []
                if len(waits) > 1:
                    changed = True
                    for w in waits[:-1]:
                        _SPLIT_CTR[0] += 1
                        nop = mybir.InstNoOp(
                            name=f"wsplit-{_SPLIT_CTR[0]}", ins=[], outs=[]
                        )
                        nop.engine = inst.engine
                        nop.sync_info = bass_rust.SyncInfo(
                            on_wait=[w], on_update=[]
                        )
                        out.append(nop)
                    si.on_wait = waits[-1:]
                out.append(inst)
            if changed:
                bb.instructions = out


def _install_fixups():
    import concourse.tile as tile

    if getattr(tile.TileContext, "_wait_split_patch", False):
        return
    tile.TileContext._wait_split_patch = True

    _orig_exit = tile.TileContext.__exit__

    def _patched_exit(self, exc_type, exc_val, exc_tb):
        r = _orig_exit(self, exc_type, exc_val, exc_tb)
        if exc_type is None:
            _split_multi_waits(self.nc)
        return r

    tile.TileContext.__exit__ = _patched_exit


def _build_program():
    _install_fixups()
    import concourse.bass as bass
    import concourse.tile as tile
    from concourse import mybir
    from concourse.masks import make_identity

    f32 = mybir.dt.float32
    f32r = mybir.dt.float32r
    bf16 = mybir.dt.bfloat16
    AF = mybir.ActivationFunctionType
    ALU = mybir.AluOpType

    nc = bass.Bass("TRN2", target_bir_lowering=False, debug=False, num_devices=NC)

    x1s = nc.dram_tensor("x1s", [MS, H], f32, kind="ExternalInput").ap()
    pros = nc.dram_tensor("pros", [MS, H], f32, kind="ExternalInput").ap()
    x2s = nc.dram_tensor("x2s", [NS, H], f32, kind="ExternalInput").ap()
    gats = nc.dram_tensor("gats", [MS, NS], f32, kind="ExternalInput").ap()
    wsum_d = nc.dram_tensor("wsum", [128, T1], f32, kind="ExternalOutput").ap()
    prow_d = nc.dram_tensor("prow", [128, T1], f32, kind="ExternalOutput").ap()
    ntsr_d = nc.dram_tensor("ntsr", [128, T1], f32, kind="ExternalOutput").ap()

    x1t_v = x1s.rearrange("(u p) h -> u p h", p=128)
    pro_v = pros.rearrange("(u p) h -> u p h", p=128)
    x2t_v = x2s.rearrange("(t p) h -> t p h", p=128)
    gat_v = gats.rearrange("(u p) (j n) -> u p j n", p=128, n=1024)

    from contextlib import ExitStack

    with tile.TileContext(nc) as tc, ExitStack() as ctx:
        singles = ctx.enter_context(tc.tile_pool(name="singles", bufs=1))
        io_pool = ctx.enter_context(tc.tile_pool(name="io", bufs=4))
        gat_pool = ctx.enter_context(tc.tile_pool(name="gat", bufs=6))
        s_pool = ctx.enter_context(tc.tile_pool(name="s", bufs=6))
        scr_pool = ctx.enter_context(tc.tile_pool(name="scr", bufs=2))
        diag_pool = ctx.enter_context(tc.tile_pool(name="diag", bufs=3))
        col_pool = ctx.enter_context(tc.tile_pool(name="cols", bufs=1))
        pt_psum = ctx.enter_context(tc.tile_pool(name="ptp", bufs=2, space="PSUM"))
        mm_psum = ctx.enter_context(tc.tile_pool(name="mmp", bufs=3, space="PSUM"))

        ident = singles.tile([128, 128], f32)
        make_identity(nc, ident)
        ln2b = singles.tile([128, 1], f32)
        nc.vector.memset(ln2b, math.log(1.0 / TAU))
        zerob = singles.tile([128, 1], f32)
        nc.vector.memset(zerob, 0.0)

        # persistent SBUF: transposed+scaled operands and accumulator columns
        x1T = singles.tile([128, T1 * 4 * 128], f32r)   # block (u,c): x1T*r1
        z2T = singles.tile([128, T2 * 4 * 128], f32r)   # block (t,c): x2T*r2
        ssq1c = col_pool.tile([128, T1], f32)
        ssq2c = col_pool.tile([128, T2], f32)
        ln1c = col_pool.tile([128, T1], f32)
        ln2c = col_pool.tile([128, T2], f32)
        r1c = col_pool.tile([128, T1], f32)
        r2c = col_pool.tile([128, T2], f32)
        ntsc = col_pool.tile([128, T1], f32)
        wacc = col_pool.tile([128, T1 * (NJ // 2)], f32)   # per (u,group) weighted
        pacc = col_pool.tile([128, T1 * (NJ // 2)], f32)   # per (u,group) plain
        wsum_sb = col_pool.tile([128, T1], f32)
        prow_sb = col_pool.tile([128, T1], f32)

        def prep(idx, src_v, n_tiles, ssqc, lnc, rc, dstT, bias_ap, extra=None):
            """Load tile idx, compute r = scale/(norm), write transposed
            scaled tile into dstT block idx."""
            xt = io_pool.tile([128, H], f32r, tag="xt")
            nc.sync.dma_start(out=xt, in_=src_v[idx].bitcast(f32r))
            scr = scr_pool.tile([128, H], f32, tag="scr")
            nc.vector.scalar_tensor_tensor(
                out=scr, in0=xt, scalar=1.0, in1=xt,
                op0=ALU.mult, op1=ALU.mult, accum_out=ssqc[:, idx : idx + 1],
            )
            if extra is not None:
                extra(xt)
            nc.scalar.activation(
                lnc[:, idx : idx + 1], ssqc[:, idx : idx + 1], AF.Ln, bias=zerob
            )
            # r = exp(-0.5*ln(ssq) + bias) = scale/sqrt(ssq)
            nc.scalar.activation(
                rc[:, idx : idx + 1], lnc[:, idx : idx + 1], AF.Exp,
                bias=bias_ap, scale=-0.5,
            )
            diag = diag_pool.tile([128, 128], f32r, tag="diag")
            nc.gpsimd.tensor_scalar_mul(diag, ident, rc[:, idx : idx + 1])
            pt = pt_psum.tile([128, 512], f32, tag="pt")
            for c in range(4):
                # scaled transpose as a real matmul: x_chunk.T @ diag(r)
                nc.tensor.matmul(
                    pt[:, c * 128 : (c + 1) * 128],
                    xt[:, c * 128 : (c + 1) * 128],
                    diag,
                    start=True,
                    stop=True,
                )
            # copy [128,512] psum -> persistent block; alternate engine
            dst = dstT[:, idx * 512 : (idx + 1) * 512]
            if idx % 2 == 0:
                nc.scalar.copy(dst, pt)
            else:
                nc.vector.tensor_copy(dst, pt)

        # x1 prep (r1 = (1/tau)/||x1_m||) + now_to_skill raw dots
        for u in range(T1):
            def nts_extra(xt, u=u):
                prot = io_pool.tile([128, H], f32, tag="prot")
                nc.sync.dma_start(out=prot, in_=pro_v[u])
                scr2 = scr_pool.tile([128, H], f32, tag="scr")
                nc.vector.scalar_tensor_tensor(
                    out=scr2, in0=prot, scalar=1.0, in1=xt,
                    op0=ALU.mult, op1=ALU.mult, accum_out=ntsc[:, u : u + 1],
                )
            prep(u, x1t_v, T1, ssq1c, ln1c, r1c, x1T, ln2b, extra=nts_extra)

        z2T_v = z2T.rearrange("p (t c x) -> p t c x", c=4, x=128)

        # main loop over 1024-wide column groups Jp; x2 prep for group Jp+0
        # is emitted just before its consumers so the scheduler overlaps the
        # next group's prep with this group's matmuls (keeps PE warm).
        NG = NJ // 2                     # 1024-wide groups (4)
        for Jp in range(NG):
            for t in range(8 * Jp, 8 * Jp + 8):
                prep(t, x2t_v, T2, ssq2c, ln2c, r2c, z2T, zerob)
            for u in range(T1):
                gt = gat_pool.tile([128, 1024], bf16, tag="gt")
                nc.gpsimd.dma_start(out=gt, in_=gat_v[u, :, Jp])
                mm = mm_psum.tile([128, 1024], f32, tag="mm")
                for h in range(2):
                    for c in range(4):
                        tt = 8 * Jp + 4 * h
                        nc.tensor.matmul(
                            mm[:, h * 512 : (h + 1) * 512],
                            x1T[:, (u * 4 + c) * 128 : (u * 4 + c + 1) * 128],
                            z2T_v[:, tt : tt + 4, c, :],
                            start=(c == 0),
                            stop=(c == 3),
                        )
                st = s_pool.tile([128, 1024], bf16, tag="st")
                col = u * NG + Jp
                nc.scalar.activation(
                    st, mm, AF.Exp, bias=zerob,
                    accum_out=pacc[:, col : col + 1],
                )
                scr3 = scr_pool.tile([128, 1024], bf16, tag="scr3")
                nc.vector.scalar_tensor_tensor(
                    out=scr3, in0=st, scalar=1.0, in1=gt,
                    op0=ALU.mult, op1=ALU.mult,
                    accum_out=wacc[:, col : col + 1],
                )

        # fold column groups per u and write outputs
        wacc_v = wacc.rearrange("p (u j) -> p u j", j=NG)
        pacc_v = pacc.rearrange("p (u j) -> p u j", j=NG)
        for u in range(T1):
            nc.vector.reduce_sum(
                wsum_sb[:, u : u + 1], wacc_v[:, u, :], axis=mybir.AxisListType.X
            )
            nc.vector.reduce_sum(
                prow_sb[:, u : u + 1], pacc_v[:, u, :], axis=mybir.AxisListType.X
            )
        nc.sync.dma_start(out=wsum_d, in_=wsum_sb)
        nc.sync.dma_start(out=prow_d, in_=prow_sb)
        nc.sync.dma_start(out=ntsr_d, in_=ntsc)

    return nc


def _get_program():
    if "nc" not in _CACHE:
        _CACHE["nc"] = _build_program()
    return _CACHE["nc"]


def kernel(x1, x2, pro_skill_embed, gat_matrix):
    from concourse.bass_utils import run_bass_kernel_spmd

    x1 = np.ascontiguousarray(np.asarray(x1, dtype=np.float32))
    x2 = np.ascontiguousarray(np.asarray(x2, dtype=np.float32))
    pro = np.ascontiguousarray(np.asarray(pro_skill_embed, dtype=np.float32))
    gat = np.asarray(gat_matrix, dtype=np.float32)

    nc = _get_program()

    in_maps = []
    for c in range(NC):
        i, j = c // GN, c % GN
        in_maps.append(
            {
                "x1s": x1[i * MS : (i + 1) * MS],
                "pros": pro[i * MS : (i + 1) * MS],
                "x2s": x2[j * NS : (j + 1) * NS],
                "gats": np.ascontiguousarray(
                    gat[i * MS : (i + 1) * MS, j * NS : (j + 1) * NS]
                ),
            }
        )

    res = run_bass_kernel_spmd(nc, in_maps, core_ids=list(range(NC)))

    w = np.zeros(N, np.float32)
    p = np.zeros(N, np.float32)
    ntsr = np.zeros(N, np.float32)
    for c, r in enumerate(res.results):
        i, j = c // GN, c % GN
        sl = slice(i * MS, (i + 1) * MS)
        w[sl] += r["wsum"].T.reshape(MS)   # [128,T1] -> m = u*128+p order
        p[sl] += r["prow"].T.reshape(MS)
        if j == 0:
            ntsr[sl] = r["ntsr"].T.reshape(MS)

    global _LAST
    _LAST = {"w": w, "p": p, "ntsr": ntsr}

    # final combine in fp32, matching reference overflow semantics
    with np.errstate(over="ignore", invalid="ignore", divide="ignore"):
        nts = np.exp((ntsr * np.float32(1.0 / TAU)).astype(np.float32))
        fenzi = nts + w
        fenmu = nts + p
        sc = -np.log(fenzi / (fenmu + np.float32(EPS_DEN)))
        out = np.mean(sc, dtype=np.float32)
    return np.asarray(out, dtype=np.float32)


# revision 14
# speedup vs baseline: 1.5874x; 1.5874x over previous
"""Trainium2 Bass kernel for nn_ContrastLoss (8-core SPMD).

Math (reference):
    z1 = x1/||x1||, z2 = x2/||x2||                    [8192, 512]
    S = exp(z1 @ z2.T / tau)                          [8192, 8192]
    nts = exp(sum(pro*x1, -1) / tau)                  [8192]
    fenzi = nts + sum(S*gat, -1); fenmu = nts + sum(S, -1)
    loss = mean(-log(fenzi / (fenmu + 1e-8)))

Sharding: 4x2 grid over (rows of x1, rows of x2). Each core computes its
[2048, 4096] block of S on the fly (never materialized), producing partial
row sums. Host sums partials over the x2 axis and does the final tiny
[8192]-row combine in float32 (faithful to the reference's fp32 overflow:
exp(dot/tau) hits inf for ~500 rows, so the reference mean is NaN, which
this kernel reproduces exactly).

On-device per core:
  - x1/x2 row norms via fused DVE tensor_tensor_reduce (sumsq)
  - r = exp(-0.5*ln(sumsq)) on ACT (ln+exp share one table set)
  - normalization folded into PE transpose-with-diag(r): x1T is scaled by
    1/(tau*||x1_m||), z2T by 1/||x2_n||, so matmul output is directly
    z1.z2/tau
  - main loop: fp32 matmul (K=512 as 4 accumulating MMs) -> ACT exp with
    accum_out (plain row sum for free) -> DVE tensor_tensor_reduce with
    gat tile (weighted row sum, fused multiply+reduce)
"""

import math

import numpy as np

N = 8192
H = 512
TAU = 0.5
EPS_DEN = 1e-8
GM, GN = 4, 2          # core grid: GM row-shards x GN col-shards
NC = GM * GN
MS = N // GM           # rows of x1 per core (2048)
NS = N // GN           # rows of x2 per core (4096)
T1 = MS // 128         # x1 tiles per core (16)
T2 = NS // 128         # x2 tiles per core (32)
NJ = NS // 512         # 512-wide n-chunks per core (8)

_CACHE = {}


_SPLIT_CTR = [0]


def _split_multi_waits(nc):
    """This container's walrus rejects any instruction carrying more than one
    semaphore wait ("Too many sync wait commands"). Tile freely attaches
    several. Move extra waits onto NoOp instructions inserted just before the
    carrier (same engine) — equivalent, since all waits are monotonic sem-ge
    conditions checked by a serial engine."""
    import bass_rust
    from concourse import mybir

    for f in nc.m.functions:
        for bb in f.blocks:
            out = []
            changed = False
            for inst in bb.instructions:
                si = getattr(inst, "sync_info", None)
                waits = list(si.on_wait) if si is not None else []
                if len(waits) > 1:
                    changed = True
                    for w in waits[:-1]:
                        _SPLIT_CTR[0] += 1
                        nop = mybir.InstNoOp(
                            name=f"wsplit-{_SPLIT_CTR[0]}", ins=[], outs=[]
                        )
                        nop.engine = inst.engine
                        nop.sync_info = bass_rust.SyncInfo(
                            on_wait=[w], on_update=[]
                        )
                        out.append(nop)
                    si.on_wait = waits[-1:]
                out.append(inst)
            if changed:
                bb.instructions = out


def _install_fixups():
    import concourse.tile as tile

    if getattr(tile.TileContext, "_wait_split_patch", False):
        return
    tile.TileContext._wait_split_patch = True

    _orig_exit = tile.TileContext.__exit__

    def _patched_exit(self, exc_type, exc_val, exc_tb):
        r = _orig_exit(self, exc_type, exc_val, exc_tb)
        if exc_type is None:
            _split_multi_waits(self.nc)
        return r

    tile.TileContext.__exit__ = _patched_exit


def _build_program():
    _install_fixups()
    import concourse.bass as bass
    import concourse.tile as tile
    from concourse import mybir
    from concourse.masks import make_identity

    f32 = mybir.dt.float32
    f32r = mybir.dt.float32r
    bf16 = mybir.dt.bfloat16
    AF = mybir.ActivationFunctionType
    ALU = mybir.AluOpType

    nc = bass.Bass("TRN2", target_bir_lowering=False, debug=False, num_devices=NC)

    x1s = nc.dram_tensor("x1s", [MS, H], f32, kind="ExternalInput").ap()
    pros = nc.dram_tensor("pros", [MS, H], f32, kind="ExternalInput").ap()
    x2s = nc.dram_tensor("x2s", [NS, H], f32, kind="ExternalInput").ap()
    gats = nc.dram_tensor("gats", [MS, NS], f32, kind="ExternalInput").ap()
    wsum_d = nc.dram_tensor("wsum", [128, T1], f32, kind="ExternalOutput").ap()
    prow_d = nc.dram_tensor("prow", [128, T1], f32, kind="ExternalOutput").ap()
    ntsr_d = nc.dram_tensor("ntsr", [128, T1], f32, kind="ExternalOutput").ap()

    x1t_v = x1s.rearrange("(u p) h -> u p h", p=128)
    pro_v = pros.rearrange("(u p) h -> u p h", p=128)
    x2t_v = x2s.rearrange("(t p) h -> t p h", p=128)
    gat_v = gats.rearrange("(u p) (j n) -> u p j n", p=128, n=1024)

    from contextlib import ExitStack

    with tile.TileContext(nc) as tc, ExitStack() as ctx:
        singles = ctx.enter_context(tc.tile_pool(name="singles", bufs=1))
        io_pool = ctx.enter_context(tc.tile_pool(name="io", bufs=6))
        gat_pool = ctx.enter_context(tc.tile_pool(name="gat", bufs=4))
        s_pool = ctx.enter_context(tc.tile_pool(name="s", bufs=4))
        scr_pool = ctx.enter_context(tc.tile_pool(name="scr", bufs=4))
        diag_pool = ctx.enter_context(tc.tile_pool(name="diag", bufs=4))
        col_pool = ctx.enter_context(tc.tile_pool(name="cols", bufs=1))
        pt_psum = ctx.enter_context(tc.tile_pool(name="ptp", bufs=2, space="PSUM"))
        mm_psum = ctx.enter_context(tc.tile_pool(name="mmp", bufs=3, space="PSUM"))

        ident = singles.tile([128, 128], f32)
        make_identity(nc, ident)
        ln2b = singles.tile([128, 1], f32)
        nc.vector.memset(ln2b, math.log(1.0 / TAU))
        zerob = singles.tile([128, 1], f32)
        nc.vector.memset(zerob, 0.0)

        # persistent SBUF: transposed+scaled operands and accumulator columns
        x1T = singles.tile([128, T1 * 4 * 128], f32r)   # block (u,c): x1T*r1
        z2T = singles.tile([128, T2 * 4 * 128], f32r)   # block (t,c): x2T*r2
        ssq1c = col_pool.tile([128, T1], f32)
        ssq2c = col_pool.tile([128, T2], f32)
        ln1c = col_pool.tile([128, T1], f32)
        ln2c = col_pool.tile([128, T2], f32)
        r1c = col_pool.tile([128, T1], f32)
        r2c = col_pool.tile([128, T2], f32)
        ntsc = col_pool.tile([128, T1], f32)
        wacc = col_pool.tile([128, T1 * (NJ // 2)], f32)   # per (u,group) weighted
        pacc = col_pool.tile([128, T1 * (NJ // 2)], f32)   # per (u,group) plain
        wsum_sb = col_pool.tile([128, T1], f32)
        prow_sb = col_pool.tile([128, T1], f32)

        def prep(idx, src_v, n_tiles, ssqc, lnc, rc, dstT, bias_ap, extra=None):
            """Load tile idx, compute r = scale/(norm), write transposed
            scaled tile into dstT block idx."""
            xt = io_pool.tile([128, H], f32r, tag="xt")
            nc.sync.dma_start(out=xt, in_=src_v[idx].bitcast(f32r))
            scr = scr_pool.tile([128, H], f32, tag="scr")
            nc.vector.scalar_tensor_tensor(
                out=scr, in0=xt, scalar=1.0, in1=xt,
                op0=ALU.mult, op1=ALU.mult, accum_out=ssqc[:, idx : idx + 1],
            )
            if extra is not None:
                extra(xt)
            nc.scalar.activation(
                lnc[:, idx : idx + 1], ssqc[:, idx : idx + 1], AF.Ln, bias=zerob
            )
            # r = exp(-0.5*ln(ssq) + bias) = scale/sqrt(ssq)
            nc.scalar.activation(
                rc[:, idx : idx + 1], lnc[:, idx : idx + 1], AF.Exp,
                bias=bias_ap, scale=-0.5,
            )
            diag = diag_pool.tile([128, 128], f32r, tag="diag")
            nc.vector.tensor_scalar_mul(diag, ident, rc[:, idx : idx + 1])
            pt = pt_psum.tile([128, 512], f32, tag="pt")
            for c in range(4):
                # scaled transpose as a real matmul: x_chunk.T @ diag(r)
                nc.tensor.matmul(
                    pt[:, c * 128 : (c + 1) * 128],
                    xt[:, c * 128 : (c + 1) * 128],
                    diag,
                    start=True,
                    stop=True,
                )
            # copy [128,512] psum -> persistent block; alternate engine
            dst = dstT[:, idx * 512 : (idx + 1) * 512]
            nc.scalar.copy(dst, pt)

        # x1 prep (r1 = (1/tau)/||x1_m||) + now_to_skill raw dots
        for u in range(T1):
            def nts_extra(xt, u=u):
                prot = io_pool.tile([128, H], f32, tag="prot")
                nc.sync.dma_start(out=prot, in_=pro_v[u])
                scr2 = scr_pool.tile([128, H], f32, tag="scr")
                nc.vector.scalar_tensor_tensor(
                    out=scr2, in0=prot, scalar=1.0, in1=xt,
                    op0=ALU.mult, op1=ALU.mult, accum_out=ntsc[:, u : u + 1],
                )
            prep(u, x1t_v, T1, ssq1c, ln1c, r1c, x1T, ln2b, extra=nts_extra)

        z2T_v = z2T.rearrange("p (t c x) -> p t c x", c=4, x=128)

        # x2 prep (r2 = 1/||x2_n||)
        for t in range(T2):
            prep(t, x2t_v, T2, ssq2c, ln2c, r2c, z2T, zerob)

        # main loop over 1024-wide column groups Jp
        NG = NJ // 2                     # 1024-wide groups (4)
        for Jp in range(NG):
            for u in range(T1):
                gt = gat_pool.tile([128, 1024], f32, tag="gt")
                nc.sync.dma_start(out=gt, in_=gat_v[u, :, Jp])
                mm = mm_psum.tile([128, 1024], f32, tag="mm")
                for h in range(2):
                    for c in range(4):
                        tt = 8 * Jp + 4 * h
                        nc.tensor.matmul(
                            mm[:, h * 512 : (h + 1) * 512],
                            x1T[:, (u * 4 + c) * 128 : (u * 4 + c + 1) * 128],
                            z2T_v[:, tt : tt + 4, c, :],
                            start=(c == 0),
                            stop=(c == 3),
                        )
                st = s_pool.tile([128, 1024], f32, tag="st")
                col = u * NG + Jp
                nc.scalar.activation(
                    st, mm, AF.Exp, bias=zerob,
                    accum_out=pacc[:, col : col + 1],
                )
                scr3 = scr_pool.tile([128, 1024], f32, tag="scr3")
                nc.vector.scalar_tensor_tensor(
                    out=scr3, in0=st, scalar=1.0, in1=gt,
                    op0=ALU.mult, op1=ALU.mult,
                    accum_out=wacc[:, col : col + 1],
                )

        # fold column groups per u and write outputs
        wacc_v = wacc.rearrange("p (u j) -> p u j", j=NG)
        pacc_v = pacc.rearrange("p (u j) -> p u j", j=NG)
        for u in range(T1):
            nc.vector.reduce_sum(
                wsum_sb[:, u : u + 1], wacc_v[:, u, :], axis=mybir.AxisListType.X
            )
            nc.vector.reduce_sum(
                prow_sb[:, u : u + 1], pacc_v[:, u, :], axis=mybir.AxisListType.X
            )
        nc.sync.dma_start(out=wsum_d, in_=wsum_sb)
        nc.sync.dma_start(out=prow_d, in_=prow_sb)
        nc.sync.dma_start(out=ntsr_d, in_=ntsc)

    return nc


def _get_program():
    if "nc" not in _CACHE:
        _CACHE["nc"] = _build_program()
    return _CACHE["nc"]


def kernel(x1, x2, pro_skill_embed, gat_matrix):
    from concourse.bass_utils import run_bass_kernel_spmd

    x1 = np.ascontiguousarray(np.asarray(x1, dtype=np.float32))
    x2 = np.ascontiguousarray(np.asarray(x2, dtype=np.float32))
    pro = np.ascontiguousarray(np.asarray(pro_skill_embed, dtype=np.float32))
    gat = np.asarray(gat_matrix, dtype=np.float32)

    nc = _get_program()

    in_maps = []
    for c in range(NC):
        i, j = c // GN, c % GN
        in_maps.append(
            {
                "x1s": x1[i * MS : (i + 1) * MS],
                "pros": pro[i * MS : (i + 1) * MS],
                "x2s": x2[j * NS : (j + 1) * NS],
                "gats": np.ascontiguousarray(
                    gat[i * MS : (i + 1) * MS, j * NS : (j + 1) * NS]
                ),
            }
        )

    res = run_bass_kernel_spmd(nc, in_maps, core_ids=list(range(NC)))

    w = np.zeros(N, np.float32)
    p = np.zeros(N, np.float32)
    ntsr = np.zeros(N, np.float32)
    for c, r in enumerate(res.results):
        i, j = c // GN, c % GN
        sl = slice(i * MS, (i + 1) * MS)
        w[sl] += r["wsum"].T.reshape(MS)   # [128,T1] -> m = u*128+p order
        p[sl] += r["prow"].T.reshape(MS)
        if j == 0:
            ntsr[sl] = r["ntsr"].T.reshape(MS)

    global _LAST
    _LAST = {"w": w, "p": p, "ntsr": ntsr}

    # final combine in fp32, matching reference overflow semantics
    with np.errstate(over="ignore", invalid="ignore", divide="ignore"):
        nts = np.exp((ntsr * np.float32(1.0 / TAU)).astype(np.float32))
        fenzi = nts + w
        fenmu = nts + p
        sc = -np.log(fenzi / (fenmu + np.float32(EPS_DEN)))
        out = np.mean(sc, dtype=np.float32)
    return np.asarray(out, dtype=np.float32)
